# revision 21
# baseline (speedup 1.0000x reference)
"""Trainium2 Bass kernel for the DeepFuzzyCMean loss.

loss = GAMMA * sum_{n,k} u[n,k]^2 * ||x[n] - v[k]||^2
     = GAMMA * ( A + sum_k c[k]*|v_k|^2 - 2*sum_{k,d} W[k,d]*v[k,d] )
  W = u2^T @ x          [K,D]
  A = sum_n s_n*|x_n|^2 = sum_k (u2^T @ x2)[k]   (x2[n] = |x_n|^2)
  c = colsum(u2) = u2^T @ ones

Wire format (fp8 e4m3, memory-bound => 1 byte/elem): two arrays,
xe = [x (128) | x2 (1) | ones (1)] = 130 B/row (norm-augmented row,
FAISS-style) and u2 = 64 B/row (pre-squared memberships). ONE accumulating
matmul per 128-row block produces all three terms at once:

    acc[K, 130] += u2_b^T @ [x_b | x2_b | 1_b]

so the device does no elementwise work at all -- only DMA + PE. fp8 matmuls
run in DoubleRow perf mode: one instruction contracts TWO 128-row blocks,
pairing block m with block m+t/2 so the k-tile-pair stride is 16B-aligned
(the walrus `s3_lw/d3_mm_dual_fp8` ISA restriction; this also forces tile
sizes to be multiples of 16 blocks). u2 is pre-scaled on host by a dynamic
power-of-two S_U (fp8 range); x2 is scaled by S_X2. All wire values are
kept <= 224 so the e4m3 / e4m3fn encodings coincide. Host combines the
per-core [K,130] partials with v in float64.

Raw-bass implementation (manual semaphores). Tapered tile schedule: big
tiles amortize per-DMA overheads (HWDGE gen is a serial resource), smaller
final tiles keep the post-stream PE tail short. Both DMAs of a tile bump
the same counting semaphore (+16 each), so PE waits for 32 regardless of
completion order. Data-parallel over N across 8 NeuronCores, host
all-reduce.
"""

import math
import sys
import types
from contextlib import ExitStack

import ml_dtypes
import numpy as np

import concourse.bass as bass
from concourse import mybir
from concourse.bass_utils import run_bass_kernel_spmd

# run_bass_kernel_spmd(trace=True) under axon imports antenv.axon_hooks,
# which this container lacks; stub it so a BASS_TRACE env var can't crash us.
try:
    import antenv.axon_hooks  # noqa: F401
except ImportError:
    try:
        import antenv

        _stub = types.ModuleType("antenv.axon_hooks")
        _stub.get_axon_ntff_profile_hook = lambda: None
        sys.modules["antenv.axon_hooks"] = _stub
        antenv.axon_hooks = _stub
    except ImportError:
        pass

GAMMA = 1e-06
N, K, D = 262144, 64, 128
NCORES = 8
N_CORE = N // NCORES
P = 128
XCOL = D          # x2 column index within the xe row
OCOL = D + 1      # ones column index
WOUT = D + 2      # xe row width / matmul rhs width ([x | x2 | 1]) = 130
S_X2 = 0.125      # fixed scale for the |x|^2 column
USCALE = 64.0     # retained for test.py compat (unused by the fp8 path)

# Tile schedule in 128-row blocks (sum = N_CORE/P = 256). Every tile is a
# multiple of 16 blocks: DoubleRow pairs (m, m+t/2) need a 16B-aligned
# k-tile-pair stride, i.e. (t/2)*130 % 16 == 0. Large tiles amortize the
# per-DMA fixed costs; the tapered tail keeps the last tile's PE+sem chain
# short.
TILES = [64, 64, 64, 32, 16, 16]

LAST_RESULTS = None
_NC_CACHE = {}


def build_nc(n_rows=N_CORE, tiles=None, num_devices=NCORES, reps=1):
    """Each tile gets a dedicated SBUF buffer (the whole per-core working set
    is ~49.7 KB/partition), so DMAs issue back-to-back with no reuse waits.
    reps>1 repeats the full sweep inside one NEFF (re-reading the same DRAM,
    cycling through the same buffers) -- used only for differential hardware
    timing."""
    if tiles is None:
        tiles = list(TILES)
    assert sum(tiles) * P == n_rows
    assert all(t % 16 == 0 for t in tiles)
    nbuf = len(tiles)
    sched = tiles * reps
    f8 = mybir.dt.float8e4
    f32 = mybir.dt.float32

    nc = bass.Bass("TRN2", num_devices=num_devices)
    xe_d = nc.dram_tensor("xe", [n_rows, WOUT], f8, kind="ExternalInput")
    u2_d = nc.dram_tensor("u2", [n_rows, K], f8, kind="ExternalInput")
    out_d = nc.dram_tensor("out", [K, WOUT], f32, kind="ExternalOutput")

    with ExitStack() as ctx:
        xt = [
            ctx.enter_context(nc.sbuf_tensor(f"xt{j}", [P, tiles[j] * WOUT], f8))
            for j in range(nbuf)
        ]
        ut = [
            ctx.enter_context(nc.sbuf_tensor(f"ut{j}", [P, tiles[j] * K], f8))
            for j in range(nbuf)
        ]
        res = ctx.enter_context(nc.sbuf_tensor("res", [K, WOUT], f32))
        acc = ctx.enter_context(nc.psum_tensor([K, WOUT], f32))

        # one counting sem per tile slot; both of the tile's DMAs add 16, so
        # a consumer waiting for 32*use is safe regardless of completion order
        s_d = [ctx.enter_context(nc.semaphore(f"s_d{j}")) for j in range(nbuf)]
        s_pe = ctx.enter_context(nc.semaphore("s_pe"))
        s_res = ctx.enter_context(nc.semaphore("s_res"))
        s_do = ctx.enter_context(nc.semaphore("s_do"))

        block = ctx.enter_context(nc.Block())

        @block.sync
        def _(sync):
            for i, t in enumerate(sched):
                j = i % nbuf
                r = (sum(sched[:i]) * P) % n_rows
                if i >= nbuf:
                    # slot reuse (reps>1 timing mode only): PE consumed the
                    # previous tile in this slot; the ordered s_d wait keeps
                    # per-sem increments monotone for the race checker.
                    sync.wait_ge(s_pe, i - nbuf + 1)
                    sync.wait_ge(s_d[j], 32 * (i // nbuf))
                x_src = xe_d[r : r + P * t, :].rearrange("(p b) c -> p (b c)", p=P)
                u_src = u2_d[r : r + P * t, :].rearrange("(p b) k -> p (b k)", p=P)
                sync.dma_start(out=xt[j][:, 0 : t * WOUT], in_=x_src).then_inc(s_d[j], 16)
                sync.dma_start(out=ut[j][:, 0 : t * K], in_=u_src).then_inc(s_d[j], 16)
            sync.wait_ge(s_res, 1)
            # every DGE needs sync info, and a then_inc with no waiter fails
            # at execution -- both the increment and the wait are load-bearing
            sync.dma_start(out=out_d[:, :], in_=res[:, :]).then_inc(s_do, 16)
            sync.wait_ge(s_do, 16)

        @block.vector
        def _(vector):
            # tail: psum -> sbuf -> (sync engine DMAs it out). DVE starts a
            # touch sooner and finishes earlier than ACT for this shape.
            vector.wait_ge(s_pe, len(sched))
            vector.tensor_copy(res[:, :], acc[:, :]).then_inc(s_res)

        @block.tensor
        def _(tensor):
            n_mm = sum(t // 2 for t in sched)
            mm = 0
            for i, t in enumerate(sched):
                j = i % nbuf
                tensor.wait_ge(s_d[j], 32 * (i // nbuf + 1))
                # split the tile into two half-tiles of t/2 blocks; DoubleRow
                # pairs block m with block m+t/2, giving a 16B-aligned k-tile
                # pair stride ((t/2)*WOUT and (t/2)*K bytes)
                x2h = xt[j][:, 0 : t * WOUT].rearrange("p (two h) -> p two h", two=2)
                u2h = ut[j][:, 0 : t * K].rearrange("p (two h) -> p two h", two=2)
                last = None
                for m in range(t // 2):
                    last = tensor.matmul(
                        acc[:, :],
                        lhsT=u2h[:, :, m * K : (m + 1) * K],
                        rhs=x2h[:, :, m * WOUT : (m + 1) * WOUT],
                        start=(mm == 0),
                        stop=(mm == n_mm - 1),
                        perf_mode=mybir.MatmulPerfMode.DoubleRow,
                    )
                    mm += 1
                last.then_inc(s_pe)

    return nc


def prep_inputs(x, u):
    """Pack xe = [x | x2*S_X2 | 1] and u2*S_U fp8 rows; returns (xe, u2, S_U)."""
    x = np.asarray(x, np.float32)
    u = np.asarray(u, np.float32)
    x2 = np.einsum("nd,nd->n", x, x, dtype=np.float64).astype(np.float32)
    u2 = u.astype(np.float64) ** 2
    u2max = float(u2.max())
    # dynamic power-of-two scale: keep u2*S_U <= 128 (fp8 e4m3 finite range
    # with margin; also keeps e4m3/e4m3fn encodings identical)
    S_U = 2.0 ** math.floor(math.log2(128.0 / max(u2max, 1e-30)))
    S_U = min(S_U, 2.0**40)
    xe = np.empty((x.shape[0], WOUT), dtype=ml_dtypes.float8_e4m3)
    xe[:, 0:D] = np.clip(x, -224.0, 224.0)
    xe[:, XCOL] = np.clip(x2 * S_X2, 0.0, 224.0)
    xe[:, OCOL] = 1.0
    u2_8 = np.clip(u2 * S_U, 0.0, 224.0).astype(ml_dtypes.float8_e4m3)
    return xe, u2_8, S_U


def combine_host(parts, v, S_U):
    """Combine per-core [K, WOUT] partials with v in float64 on the host."""
    acc = np.zeros((K, WOUT), np.float64)
    for p in parts:
        acc += np.asarray(p, np.float64)
    W = acc[:, 0:D] / S_U
    A = acc[:, XCOL].sum() / (S_U * S_X2)
    c = acc[:, OCOL] / S_U
    v64 = np.asarray(v, np.float64)
    v2 = (v64 * v64).sum(axis=1)
    loss = A + (v2 * c).sum() - 2.0 * (W * v64).sum()
    return np.asarray(GAMMA * loss, dtype=np.float32)


def kernel(x, u, v):
    global LAST_RESULTS
    x = np.asarray(x)
    u = np.asarray(u)
    assert x.shape == (N, D) and u.shape == (N, K)
    xe, u2_8, S_U = prep_inputs(x, u)

    if "nc" not in _NC_CACHE:
        _NC_CACHE["nc"] = build_nc()
    nc = _NC_CACHE["nc"]

    in_maps = [
        {
            "xe": xe[c * N_CORE : (c + 1) * N_CORE],
            "u2": u2_8[c * N_CORE : (c + 1) * N_CORE],
        }
        for c in range(NCORES)
    ]
    LAST_RESULTS = run_bass_kernel_spmd(nc, in_maps, list(range(NCORES)))
    return combine_host([r["out"] for r in LAST_RESULTS.results], v, S_U)


# revision 22
# speedup vs baseline: 1.0025x; 1.0025x over previous
"""Trainium2 Bass kernel for the DeepFuzzyCMean loss.

loss = GAMMA * sum_{n,k} u[n,k]^2 * ||x[n] - v[k]||^2
     = GAMMA * ( A + sum_k c[k]*|v_k|^2 - 2*sum_{k,d} W[k,d]*v[k,d] )
  W = u2^T @ x          [K,D]
  A = sum_n s_n*|x_n|^2 = sum_k (u2^T @ x2)[k]   (x2[n] = |x_n|^2)
  c = colsum(u2) = u2^T @ ones

Wire format (fp8 e4m3, memory-bound => 1 byte/elem): two arrays,
xe = [x (128) | x2 (1) | ones (1)] = 130 B/row (norm-augmented row,
FAISS-style) and u2 = 64 B/row (pre-squared memberships). ONE accumulating
matmul per 128-row block produces all three terms at once:

    acc[K, 130] += u2_b^T @ [x_b | x2_b | 1_b]

so the device does no elementwise work at all -- only DMA + PE. fp8 matmuls
run in DoubleRow perf mode: one instruction contracts TWO 128-row blocks,
pairing block m with block m+t/2 so the k-tile-pair stride is 16B-aligned
(the walrus `s3_lw/d3_mm_dual_fp8` ISA restriction; this also forces tile
sizes to be multiples of 16 blocks). u2 is pre-scaled on host by a dynamic
power-of-two S_U (fp8 range); x2 is scaled by S_X2. All wire values are
kept <= 224 so the e4m3 / e4m3fn encodings coincide. Host combines the
per-core [K,130] partials with v in float64.

Raw-bass implementation (manual semaphores). Tapered tile schedule: big
tiles amortize per-DMA overheads (HWDGE gen is a serial resource), smaller
final tiles keep the post-stream PE tail short. Both DMAs of a tile bump
the same counting semaphore (+16 each), so PE waits for 32 regardless of
completion order. Data-parallel over N across 8 NeuronCores, host
all-reduce.
"""

import math
import sys
import types
from contextlib import ExitStack

import ml_dtypes
import numpy as np

import concourse.bass as bass
from concourse import mybir
from concourse.bass_utils import run_bass_kernel_spmd

# run_bass_kernel_spmd(trace=True) under axon imports antenv.axon_hooks,
# which this container lacks; stub it so a BASS_TRACE env var can't crash us.
try:
    import antenv.axon_hooks  # noqa: F401
except ImportError:
    try:
        import antenv

        _stub = types.ModuleType("antenv.axon_hooks")
        _stub.get_axon_ntff_profile_hook = lambda: None
        sys.modules["antenv.axon_hooks"] = _stub
        antenv.axon_hooks = _stub
    except ImportError:
        pass

GAMMA = 1e-06
N, K, D = 262144, 64, 128
NCORES = 8
N_CORE = N // NCORES
P = 128
XCOL = D          # x2 column index within the xe row
OCOL = D + 1      # ones column index
WOUT = D + 2      # xe row width / matmul rhs width ([x | x2 | 1]) = 130
S_X2 = 0.125      # fixed scale for the |x|^2 column
USCALE = 64.0     # retained for test.py compat (unused by the fp8 path)

# Tile schedule in 128-row blocks (sum = N_CORE/P = 256). Every tile is a
# multiple of 16 blocks: DoubleRow pairs (m, m+t/2) need a 16B-aligned
# k-tile-pair stride, i.e. (t/2)*130 % 16 == 0. Large tiles amortize the
# per-DMA fixed costs; the tapered tail keeps the last tile's PE+sem chain
# short.
TILES = [64, 64, 64, 32, 16, 16]

LAST_RESULTS = None
_NC_CACHE = {}


def build_nc(n_rows=N_CORE, tiles=None, num_devices=NCORES, reps=1):
    """Each tile gets a dedicated SBUF buffer (the whole per-core working set
    is ~49.7 KB/partition), so DMAs issue back-to-back with no reuse waits.
    reps>1 repeats the full sweep inside one NEFF (re-reading the same DRAM,
    cycling through the same buffers) -- used only for differential hardware
    timing."""
    if tiles is None:
        tiles = list(TILES)
    assert sum(tiles) * P == n_rows
    assert all(t % 16 == 0 for t in tiles)
    nbuf = len(tiles)
    sched = tiles * reps
    f8 = mybir.dt.float8e4
    f32 = mybir.dt.float32

    # monotonic_sem_count=0: we use no MonotonicSemaphores; skipping their
    # reservation trims the Pool-engine preamble that gates the start barrier
    nc = bass.Bass("TRN2", num_devices=num_devices, monotonic_sem_count=0)
    xe_d = nc.dram_tensor("xe", [n_rows, WOUT], f8, kind="ExternalInput")
    u2_d = nc.dram_tensor("u2", [n_rows, K], f8, kind="ExternalInput")
    out_d = nc.dram_tensor("out", [K, WOUT], f32, kind="ExternalOutput")

    with ExitStack() as ctx:
        xt = [
            ctx.enter_context(nc.sbuf_tensor(f"xt{j}", [P, tiles[j] * WOUT], f8))
            for j in range(nbuf)
        ]
        ut = [
            ctx.enter_context(nc.sbuf_tensor(f"ut{j}", [P, tiles[j] * K], f8))
            for j in range(nbuf)
        ]
        res = ctx.enter_context(nc.sbuf_tensor("res", [K, WOUT], f32))
        acc = ctx.enter_context(nc.psum_tensor([K, WOUT], f32))

        # one counting sem per tile slot; both of the tile's DMAs add 16, so
        # a consumer waiting for 32*use is safe regardless of completion order
        s_d = [ctx.enter_context(nc.semaphore(f"s_d{j}")) for j in range(nbuf)]
        s_pe = ctx.enter_context(nc.semaphore("s_pe"))
        s_res = ctx.enter_context(nc.semaphore("s_res"))
        s_do = ctx.enter_context(nc.semaphore("s_do"))

        block = ctx.enter_context(nc.Block())

        @block.sync
        def _(sync):
            for i, t in enumerate(sched):
                j = i % nbuf
                r = (sum(sched[:i]) * P) % n_rows
                if i >= nbuf:
                    # slot reuse (reps>1 timing mode only): PE consumed the
                    # previous tile in this slot; the ordered s_d wait keeps
                    # per-sem increments monotone for the race checker.
                    sync.wait_ge(s_pe, i - nbuf + 1)
                    sync.wait_ge(s_d[j], 32 * (i // nbuf))
                x_src = xe_d[r : r + P * t, :].rearrange("(p b) c -> p (b c)", p=P)
                u_src = u2_d[r : r + P * t, :].rearrange("(p b) k -> p (b k)", p=P)
                sync.dma_start(out=xt[j][:, 0 : t * WOUT], in_=x_src).then_inc(s_d[j], 16)
                sync.dma_start(out=ut[j][:, 0 : t * K], in_=u_src).then_inc(s_d[j], 16)
            sync.wait_ge(s_res, 1)
            # every DGE needs sync info, and a then_inc with no waiter fails
            # at execution -- both the increment and the wait are load-bearing
            sync.dma_start(out=out_d[:, :], in_=res[:, :]).then_inc(s_do, 16)
            sync.wait_ge(s_do, 16)

        @block.vector
        def _(vector):
            # tail: psum -> sbuf -> (sync engine DMAs it out). DVE starts a
            # touch sooner and finishes earlier than ACT for this shape.
            vector.wait_ge(s_pe, len(sched))
            vector.tensor_copy(res[:, :], acc[:, :]).then_inc(s_res)

        @block.tensor
        def _(tensor):
            n_mm = sum(t // 2 for t in sched)
            mm = 0
            for i, t in enumerate(sched):
                j = i % nbuf
                tensor.wait_ge(s_d[j], 32 * (i // nbuf + 1))
                # split the tile into two half-tiles of t/2 blocks; DoubleRow
                # pairs block m with block m+t/2, giving a 16B-aligned k-tile
                # pair stride ((t/2)*WOUT and (t/2)*K bytes)
                x2h = xt[j][:, 0 : t * WOUT].rearrange("p (two h) -> p two h", two=2)
                u2h = ut[j][:, 0 : t * K].rearrange("p (two h) -> p two h", two=2)
                last = None
                for m in range(t // 2):
                    last = tensor.matmul(
                        acc[:, :],
                        lhsT=u2h[:, :, m * K : (m + 1) * K],
                        rhs=x2h[:, :, m * WOUT : (m + 1) * WOUT],
                        start=(mm == 0),
                        stop=(mm == n_mm - 1),
                        perf_mode=mybir.MatmulPerfMode.DoubleRow,
                    )
                    mm += 1
                last.then_inc(s_pe)

    return nc


def prep_inputs(x, u):
    """Pack xe = [x | x2*S_X2 | 1] and u2*S_U fp8 rows; returns (xe, u2, S_U)."""
    x = np.asarray(x, np.float32)
    u = np.asarray(u, np.float32)
    x2 = np.einsum("nd,nd->n", x, x, dtype=np.float64).astype(np.float32)
    u2 = u.astype(np.float64) ** 2
    u2max = float(u2.max())
    # dynamic power-of-two scale: keep u2*S_U <= 128 (fp8 e4m3 finite range
    # with margin; also keeps e4m3/e4m3fn encodings identical)
    S_U = 2.0 ** math.floor(math.log2(128.0 / max(u2max, 1e-30)))
    S_U = min(S_U, 2.0**40)
    xe = np.empty((x.shape[0], WOUT), dtype=ml_dtypes.float8_e4m3)
    xe[:, 0:D] = np.clip(x, -224.0, 224.0)
    xe[:, XCOL] = np.clip(x2 * S_X2, 0.0, 224.0)
    xe[:, OCOL] = 1.0
    u2_8 = np.clip(u2 * S_U, 0.0, 224.0).astype(ml_dtypes.float8_e4m3)
    return xe, u2_8, S_U


def combine_host(parts, v, S_U):
    """Combine per-core [K, WOUT] partials with v in float64 on the host."""
    acc = np.zeros((K, WOUT), np.float64)
    for p in parts:
        acc += np.asarray(p, np.float64)
    W = acc[:, 0:D] / S_U
    A = acc[:, XCOL].sum() / (S_U * S_X2)
    c = acc[:, OCOL] / S_U
    v64 = np.asarray(v, np.float64)
    v2 = (v64 * v64).sum(axis=1)
    loss = A + (v2 * c).sum() - 2.0 * (W * v64).sum()
    return np.asarray(GAMMA * loss, dtype=np.float32)


def kernel(x, u, v):
    global LAST_RESULTS
    x = np.asarray(x)
    u = np.asarray(u)
    assert x.shape == (N, D) and u.shape == (N, K)
    xe, u2_8, S_U = prep_inputs(x, u)

    if "nc" not in _NC_CACHE:
        _NC_CACHE["nc"] = build_nc()
    nc = _NC_CACHE["nc"]

    in_maps = [
        {
            "xe": xe[c * N_CORE : (c + 1) * N_CORE],
            "u2": u2_8[c * N_CORE : (c + 1) * N_CORE],
        }
        for c in range(NCORES)
    ]
    LAST_RESULTS = run_bass_kernel_spmd(nc, in_maps, list(range(NCORES)))
    return combine_host([r["out"] for r in LAST_RESULTS.results], v, S_U)
